# revision 1
# baseline (speedup 1.0000x reference)
"""Grouped-Query Attention kernel for Trainium2 (8 NeuronCores, SPMD).

Problem: x [4, 4096, 512] fp32, per-group Dense Q/K/V (G=4 groups of 128
features), full softmax attention within each (batch, group) pair, output
re-concatenated to [4, 4096, 512].

Sharding: B*G = 16 fully independent attention problems -> 2 per core.
Per core, per pair, everything stays on-chip (SBUF 24MB):
  - load xg [4096, 128] fp32, cast bf16, PE-transpose to xgT [d, t]
  - Q^T = Wq^T xg^T, K^T likewise (bias folded in), V natural [t, e]
  - scores computed TRANSPOSED: S^T[ts, tq] = K Q^T so that the exp'd
    probabilities land directly in the layout attn@V needs as rhs
    (contraction dim ts on partitions) -- no transpose of the TxT matrix.
  - exp via ScalarE with the 1/sqrt(gs) scale folded into ACT's free affine
  - softmax denominator via an extra ones-matmul pass (all-rows-equal
    accumulate), out^T accumulated over ts chunks in PSUM
  - epilogue: reciprocal, normalize, +bv, PE-transpose back to natural
Compute dtype bf16 (fp32 accumulation in PSUM).
"""

import os
import sys

sys.path.insert(0, "/opt/trn_rl_repo")

import numpy as np

import concourse.bass as bass
import concourse.mybir as mybir
import concourse.tile as tile
from concourse.masks import make_identity

B, T, F, G = 4, 4096, 512, 4
GS = F // G  # 128
N_CORES = 8
PAIRS_PER_CORE = (B * G) // N_CORES  # 2
TQ_MACRO = 1024  # query tile width per softmax/psum round
N_MACROS = T // TQ_MACRO  # 4
N_CHUNKS = T // 128  # 32 key/time chunks
INV_SCALE = float(1.0 / (np.sqrt(np.float32(GS)) + 1e-9))

FP32 = mybir.dt.float32
BF16 = mybir.dt.bfloat16

_NC_CACHE = None
_LAST_IN_MAPS = None


def _split_multi_waits(nc):
    """Walrus codegen rejects instructions carrying more than one semaphore
    wait on several instruction structs (DMA DIRECT2D, tensor_scalar, LDW).
    Hoist all-but-the-last wait of any multi-wait instruction onto same-engine
    NoOps inserted immediately before it: the sequencer executes them in
    order, so the gating semantics are identical."""
    n_split = 0
    for func in nc.m.functions:
        for block in func.blocks:
            new = []
            for inst in block.instructions:
                si = inst.sync_info
                waits = list(si.on_wait) if (si is not None and si.on_wait) else []
                if len(waits) > 1:
                    for w in waits[:-1]:
                        nop = mybir.InstNoOp(
                            name=nc.get_next_instruction_name(), ins=[], outs=[]
                        )
                        nop.engine = inst.engine
                        nop.sync_info = mybir.SyncInfo(on_wait=[w], on_update=[])
                        new.append(nop)
                        n_split += 1
                    inst.sync_info = mybir.SyncInfo(
                        on_wait=[waits[-1]],
                        on_update=list(si.on_update) if si.on_update else [],
                    )
                new.append(inst)
            block.instructions = new
    return n_split


def build_nc():
    nc = bass.Bass()

    ins = []
    outs = []
    for i in range(PAIRS_PER_CORE):
        ins.append(
            dict(
                x=nc.declare_dram_parameter(f"x{i}", [T, GS], FP32, isOutput=False),
                wq=nc.declare_dram_parameter(f"wq{i}", [GS, GS], FP32, isOutput=False),
                wk=nc.declare_dram_parameter(f"wk{i}", [GS, GS], FP32, isOutput=False),
                wv=nc.declare_dram_parameter(f"wv{i}", [GS, GS], FP32, isOutput=False),
                bq=nc.declare_dram_parameter(f"bq{i}", [1, GS], FP32, isOutput=False),
                bk=nc.declare_dram_parameter(f"bk{i}", [1, GS], FP32, isOutput=False),
                bv=nc.declare_dram_parameter(f"bv{i}", [1, GS], FP32, isOutput=False),
            )
        )
        outs.append(nc.declare_dram_parameter(f"y{i}", [T, GS], FP32, isOutput=True))

    with tile.TileContext(nc) as tc:
        with (
            tc.tile_pool(name="consts", bufs=1) as consts,
            tc.tile_pool(name="bigsb", bufs=2) as bigsb,  # per-pair persistent
            tc.tile_pool(name="pt", bufs=4) as ptpool,  # exp'd prob chunks
            tc.tile_pool(name="epi", bufs=2) as epi,  # epilogue sbuf tiles
            tc.tile_pool(name="ps_s", bufs=2, space="PSUM") as ps_s,  # scores
            tc.tile_pool(name="ps_o", bufs=1, space="PSUM") as ps_o,  # out^T
            tc.tile_pool(name="ps_d", bufs=1, space="PSUM") as ps_d,  # denom
        ):
            ident_bf = consts.tile([128, 128], BF16)
            make_identity(nc, ident_bf)
            ident_f = consts.tile([128, 128], FP32)
            make_identity(nc, ident_f)
            ones_bf = consts.tile([128, 128], BF16)
            nc.vector.memset(ones_bf, 1.0)

            for i in range(PAIRS_PER_CORE):
                p = ins[i]
                # ---------------- prologue: load + QKV ----------------
                xg_f = bigsb.tile([128, N_CHUNKS, 128], FP32, tag="xg_f")
                nc.sync.dma_start(
                    out=xg_f, in_=p["x"][:, :].rearrange("(c p) d -> p c d", p=128)
                )
                xg_b = bigsb.tile([128, N_CHUNKS, 128], BF16, tag="xg_b")
                nc.vector.tensor_copy(xg_b, xg_f)

                # weights + biases
                w_bf = {}
                for nm in ("wq", "wk", "wv"):
                    wf = epi.tile([128, 128], FP32, tag=f"wf{nm}{i}")
                    nc.gpsimd.dma_start(out=wf, in_=p[nm][:, :])
                    wb = consts.tile([128, 128], BF16, tag=f"{nm}{i}")
                    nc.vector.tensor_copy(wb, wf)
                    w_bf[nm] = wb
                b_col = {}
                for nm in ("bq", "bk", "bv"):
                    bc = consts.tile([128, 1], FP32, tag=f"{nm}{i}")
                    nc.gpsimd.dma_start(
                        out=bc, in_=p[nm][:, :].rearrange("o d -> d o")
                    )
                    b_col[nm] = bc
                bvb = consts.tile([128, 128], FP32, tag=f"bvb{i}")
                _bv = p["bv"][:, :]
                nc.gpsimd.dma_start(
                    out=bvb,
                    in_=bass.AP(tensor=_bv.tensor, offset=_bv.offset,
                                ap=[[0, 128]] + list(_bv.ap[1:])),
                )

                # xgT [d, t] bf16 via PE transpose of 32 chunks
                xgT = bigsb.tile([128, T], BF16, tag="xgT")
                for c in range(N_CHUNKS):
                    pst = ps_s.tile([128, 128], BF16, tag="sc")
                    nc.tensor.transpose(pst, xg_b[:, c, :], ident_bf)
                    nc.vector.tensor_copy(xgT[:, c * 128 : (c + 1) * 128], pst)

                # Q^T/K^T [e, t] bf16 (bias added), V^T -> V natural
                qt = bigsb.tile([128, T], BF16, tag="qt")
                kt = bigsb.tile([128, T], BF16, tag="kt")
                vt = bigsb.tile([128, T], BF16, tag="vt")
                for dst, wname, bname in (
                    (qt, "wq", "bq"),
                    (kt, "wk", "bk"),
                    (vt, "wv", None),
                ):
                    for j in range(T // TQ_MACRO):
                        psq = ps_s.tile([128, TQ_MACRO], FP32, tag="sc")
                        for h in range(TQ_MACRO // 512):
                            sl = slice(h * 512, (h + 1) * 512)
                            tsl = slice(j * TQ_MACRO + h * 512, j * TQ_MACRO + (h + 1) * 512)
                            nc.tensor.matmul(
                                psq[:, sl], w_bf[wname], xgT[:, tsl], start=True, stop=True
                            )
                        dsl = slice(j * TQ_MACRO, (j + 1) * TQ_MACRO)
                        if bname is not None:
                            nc.vector.tensor_scalar_add(dst[:, dsl], psq, b_col[bname])
                        else:
                            nc.vector.tensor_copy(dst[:, dsl], psq)

                v_nat = bigsb.tile([128, N_CHUNKS, 128], BF16, tag="v_nat")
                for c in range(N_CHUNKS):
                    pst = ps_s.tile([128, 128], BF16, tag="sc")
                    nc.tensor.transpose(pst, vt[:, c * 128 : (c + 1) * 128], ident_bf)
                    nc.vector.tensor_copy(v_nat[:, c, :], pst)

                # ---------------- attention macros ----------------
                for m in range(N_MACROS):
                    tq0 = m * TQ_MACRO
                    ps_out = ps_o.tile([128, TQ_MACRO], FP32)
                    ps_den = ps_d.tile([128, TQ_MACRO], FP32)
                    for c in range(N_CHUNKS):
                        ksl = kt[:, c * 128 : (c + 1) * 128]
                        ps_sc = ps_s.tile([128, TQ_MACRO], FP32, tag="sc")
                        for h in range(TQ_MACRO // 512):
                            sl = slice(h * 512, (h + 1) * 512)
                            qsl = slice(tq0 + h * 512, tq0 + (h + 1) * 512)
                            nc.tensor.matmul(
                                ps_sc[:, sl], ksl, qt[:, qsl], start=True, stop=True
                            )
                        pt = ptpool.tile([128, TQ_MACRO], BF16)
                        nc.scalar.activation(
                            pt, ps_sc, mybir.ActivationFunctionType.Exp, scale=INV_SCALE
                        )
                        first, last = c == 0, c == N_CHUNKS - 1
                        for h in range(TQ_MACRO // 512):
                            sl = slice(h * 512, (h + 1) * 512)
                            nc.tensor.matmul(
                                ps_out[:, sl], v_nat[:, c, :], pt[:, sl],
                                start=first, stop=last,
                            )
                            nc.tensor.matmul(
                                ps_den[:, sl], ones_bf, pt[:, sl],
                                start=first, stop=last,
                            )
                    recip = epi.tile([128, TQ_MACRO], FP32, tag="recip")
                    nc.vector.reciprocal(recip, ps_den)
                    onorm = epi.tile([128, TQ_MACRO], FP32, tag="onorm")
                    nc.vector.tensor_mul(onorm, ps_out, recip)
                    nc.vector.tensor_scalar_add(onorm, onorm, b_col["bv"])
                    onat = epi.tile([128, TQ_MACRO // 128, 128], FP32, tag="onat")
                    for j in range(TQ_MACRO // 128):
                        pst = ps_s.tile([128, 128], FP32, tag="sc")
                        nc.tensor.transpose(pst, onorm[:, j * 128 : (j + 1) * 128], ident_f)
                        nc.vector.tensor_copy(onat[:, j, :], pst)
                    nc.sync.dma_start(
                        out=outs[i][tq0 : tq0 + TQ_MACRO, :].rearrange(
                            "(c p) d -> p c d", p=128
                        ),
                        in_=onat,
                    )
    _split_multi_waits(nc)
    return nc


def _get_nc():
    global _NC_CACHE
    if _NC_CACHE is None:
        _NC_CACHE = build_nc()
    return _NC_CACHE


def kernel(**inputs: np.ndarray) -> np.ndarray:
    x = np.ascontiguousarray(inputs["x"], dtype=np.float32)
    Wq = np.asarray(inputs["Wq"], dtype=np.float32)
    Wk = np.asarray(inputs["Wk"], dtype=np.float32)
    Wv = np.asarray(inputs["Wv"], dtype=np.float32)
    bq = np.asarray(inputs["bq"], dtype=np.float32)
    bk = np.asarray(inputs["bk"], dtype=np.float32)
    bv = np.asarray(inputs["bv"], dtype=np.float32)

    nc = _get_nc()

    in_maps = []
    for core in range(N_CORES):
        m = {}
        for i in range(PAIRS_PER_CORE):
            pair = core * PAIRS_PER_CORE + i
            b, g = pair // G, pair % G
            sl = slice(g * GS, (g + 1) * GS)
            m[f"x{i}"] = np.ascontiguousarray(x[b, :, sl])
            m[f"wq{i}"] = np.ascontiguousarray(Wq[g])
            m[f"wk{i}"] = np.ascontiguousarray(Wk[g])
            m[f"wv{i}"] = np.ascontiguousarray(Wv[g])
            m[f"bq{i}"] = np.ascontiguousarray(bq[g].reshape(1, GS))
            m[f"bk{i}"] = np.ascontiguousarray(bk[g].reshape(1, GS))
            m[f"bv{i}"] = np.ascontiguousarray(bv[g].reshape(1, GS))
        in_maps.append(m)

    global _LAST_IN_MAPS
    _LAST_IN_MAPS = in_maps

    from concourse.bass_utils import run_bass_kernel_spmd

    res = run_bass_kernel_spmd(nc, in_maps, list(range(N_CORES)))

    y = np.empty((B, T, F), dtype=np.float32)
    for core in range(N_CORES):
        for i in range(PAIRS_PER_CORE):
            pair = core * PAIRS_PER_CORE + i
            b, g = pair // G, pair % G
            y[b, :, g * GS : (g + 1) * GS] = res.results[core][f"y{i}"]
    return y



# revision 4
# speedup vs baseline: 1.4195x; 1.4195x over previous
"""Grouped-Query Attention kernel for Trainium2 (8 NeuronCores, SPMD).

Problem: x [4, 4096, 512] fp32, per-group Dense Q/K/V (G=4 groups of 128
features), full softmax attention within each (batch, group) pair, output
re-concatenated to [4, 4096, 512].

Sharding: B*G = 16 fully independent attention problems -> 2 per core.
Per core, per pair, everything stays on-chip (SBUF 24MB).

Pipeline (v2):
  - x loaded fp32 (chunked DMA), cast bf16, PE-transposed to xgT [d, t]
  - Q^T/K^T = W^T xgT (bias folded) bf16; V natural [t, e] via
    xgT-chunk-stationary matmuls, quantized to fp8 e4m3
  - scores computed TRANSPOSED per 128-key chunk: S^T[ts, tq] = K_c Q^T
    (bf16 matmuls)
  - exp via ScalarE ACT with scale=1/sqrt(gs) and bias=-3 folded in
    (the e^-3 shift cancels in softmax; keeps exp < 240 = trn-e4m3 max),
    output DIRECTLY in fp8 e4m3 into chunk-pair tiles [128, 2, 1024]
  - P@V and the ones-denominator matmul run in fp8 DoubleRow perf mode
    (K=256: two key chunks per pass -> half the streamed columns)
  - epilogue: reciprocal, normalize (+bq... bv col), PE-transpose back
    (bf16), DMA out fp32
Compute dtype bf16 for scores, fp8 for probabilities/V (fp32 PSUM acc).
"""

import os
import sys

sys.path.insert(0, "/opt/trn_rl_repo")

import numpy as np

import concourse.bass as bass
import concourse.mybir as mybir
import concourse.tile as tile
from concourse.masks import make_identity

B, T, F, G = 4, 4096, 512, 4
GS = F // G  # 128
N_CORES = 8
PAIRS_PER_CORE = (B * G) // N_CORES  # 2
TQ_MACRO = 1024  # query tile width per softmax/psum round
N_MACROS = T // TQ_MACRO  # 4
N_CHUNKS = T // 128  # 32 key/time chunks
N_CPAIRS = N_CHUNKS // 2  # 16 chunk pairs (DoubleRow K=256)
INV_SCALE = float(1.0 / (np.sqrt(np.float32(GS)) + 1e-9))
EXP_SHIFT = -3.0  # exp(s - 3): cancels in softmax, keeps max < 240

FP32 = mybir.dt.float32
BF16 = mybir.dt.bfloat16
FP8 = mybir.dt.float8e4
DR = mybir.MatmulPerfMode.DoubleRow

_NC_CACHE = None
_LAST_IN_MAPS = None


def _split_multi_waits(nc):
    """Walrus codegen rejects instructions carrying more than one semaphore
    wait on several instruction structs (DMA DIRECT2D, tensor_scalar, LDW).
    Hoist all-but-the-last wait of any multi-wait instruction onto same-engine
    NoOps inserted immediately before it: the sequencer executes them in
    order, so the gating semantics are identical."""
    n_split = 0
    for func in nc.m.functions:
        for block in func.blocks:
            new = []
            for inst in block.instructions:
                si = inst.sync_info
                waits = list(si.on_wait) if (si is not None and si.on_wait) else []
                if len(waits) > 1:
                    for w in waits[:-1]:
                        nop = mybir.InstNoOp(
                            name=nc.get_next_instruction_name(), ins=[], outs=[]
                        )
                        nop.engine = inst.engine
                        nop.sync_info = mybir.SyncInfo(on_wait=[w], on_update=[])
                        new.append(nop)
                        n_split += 1
                    inst.sync_info = mybir.SyncInfo(
                        on_wait=[waits[-1]],
                        on_update=list(si.on_update) if si.on_update else [],
                    )
                new.append(inst)
            block.instructions = new
    return n_split


def build_nc():
    nc = bass.Bass()

    ins = []
    outs = []
    for i in range(PAIRS_PER_CORE):
        ins.append(
            dict(
                x=nc.declare_dram_parameter(f"x{i}", [T, GS], FP32, isOutput=False),
                wq=nc.declare_dram_parameter(f"wq{i}", [GS, GS], FP32, isOutput=False),
                wk=nc.declare_dram_parameter(f"wk{i}", [GS, GS], FP32, isOutput=False),
                wv=nc.declare_dram_parameter(f"wv{i}", [GS, GS], FP32, isOutput=False),
                bq=nc.declare_dram_parameter(f"bq{i}", [1, GS], FP32, isOutput=False),
                bk=nc.declare_dram_parameter(f"bk{i}", [1, GS], FP32, isOutput=False),
                bv=nc.declare_dram_parameter(f"bv{i}", [1, GS], FP32, isOutput=False),
            )
        )
        outs.append(nc.declare_dram_parameter(f"y{i}", [T, GS], FP32, isOutput=True))

    with tile.TileContext(nc) as tc:
        with (
            tc.tile_pool(name="consts", bufs=1) as consts,
            tc.tile_pool(name="bigsb", bufs=2) as bigsb,  # per-pair persistent
            tc.tile_pool(name="pt", bufs=3) as ptpool,  # exp'd prob chunk-pairs
            tc.tile_pool(name="epi", bufs=2) as epi,  # epilogue sbuf tiles
            tc.tile_pool(name="ps_s", bufs=2, space="PSUM") as ps_s,  # scores
            tc.tile_pool(name="ps_o", bufs=1, space="PSUM") as ps_o,  # out^T
            tc.tile_pool(name="ps_d", bufs=1, space="PSUM") as ps_d,  # denom
        ):
            ident_bf = consts.tile([128, 128], BF16)
            make_identity(nc, ident_bf)
            ones2_f8 = consts.tile([128, 2, 128], FP8)
            nc.vector.memset(ones2_f8, 1.0)
            exp_bias = consts.tile([128, 1], FP32)
            nc.vector.memset(exp_bias, EXP_SHIFT)

            # ---- prefetch: x + weights for BOTH pairs up-front ----
            xg_fs = []
            w_sb = []
            for i in range(PAIRS_PER_CORE):
                p = ins[i]
                xg_f = bigsb.tile([128, N_CHUNKS, 128], FP32, tag="xg_f")
                for d in range(4):
                    nc.sync.dma_start(
                        out=xg_f[:, d * 8 : (d + 1) * 8, :],
                        in_=p["x"][d * 1024 : (d + 1) * 1024, :].rearrange(
                            "(c p) d -> p c d", p=128
                        ),
                    )
                xg_fs.append(xg_f)
                wd = {}
                for nm in ("wq", "wk", "wv"):
                    wf = epi.tile([128, 128], FP32, tag=f"wf{nm}{i}")
                    nc.gpsimd.dma_start(out=wf, in_=p[nm][:, :])
                    wb = consts.tile([128, 128], BF16, tag=f"{nm}{i}")
                    nc.vector.tensor_copy(wb, wf)
                    wd[nm] = wb
                for nm in ("bq", "bk", "bv"):
                    bc = consts.tile([128, 1], FP32, tag=f"{nm}{i}")
                    nc.gpsimd.dma_start(
                        out=bc, in_=p[nm][:, :].rearrange("o d -> d o")
                    )
                    wd[nm] = bc
                w_sb.append(wd)

            for i in range(PAIRS_PER_CORE):
                w_bf = w_sb[i]
                # ---------------- prologue ----------------
                xg_f = xg_fs[i]
                xg_b = bigsb.tile([128, N_CHUNKS, 128], BF16, tag="xg_b")
                for d in range(4):
                    nc.vector.tensor_copy(
                        xg_b[:, d * 8 : (d + 1) * 8, :], xg_f[:, d * 8 : (d + 1) * 8, :]
                    )

                # xgT [d, t] bf16 via PE transpose of 32 chunks
                xgT = bigsb.tile([128, T], BF16, tag="xgT")
                for c in range(N_CHUNKS):
                    pst = ps_s.tile([128, 128], BF16, tag="sc")
                    nc.tensor.transpose(pst, xg_b[:, c, :], ident_bf)
                    nc.vector.tensor_copy(xgT[:, c * 128 : (c + 1) * 128], pst)

                # Q^T/K^T [e, t] bf16 (bias added)
                qt = bigsb.tile([128, T], BF16, tag="qt")
                kt = bigsb.tile([128, T], BF16, tag="kt")
                for dst, wname, bname in ((qt, "wq", "bq"), (kt, "wk", "bk")):
                    for j in range(T // TQ_MACRO):
                        psq = ps_s.tile([128, TQ_MACRO], FP32, tag="sc")
                        for h in range(TQ_MACRO // 512):
                            sl = slice(h * 512, (h + 1) * 512)
                            tsl = slice(
                                j * TQ_MACRO + h * 512, j * TQ_MACRO + (h + 1) * 512
                            )
                            nc.tensor.matmul(
                                psq[:, sl], w_bf[wname], xgT[:, tsl],
                                start=True, stop=True,
                            )
                        dsl = slice(j * TQ_MACRO, (j + 1) * TQ_MACRO)
                        nc.vector.tensor_scalar_add(dst[:, dsl], psq, w_bf[bname])

                # V natural [t, e] fp8 via xgT-chunk-stationary matmuls
                v8 = bigsb.tile([128, N_CHUNKS, 128], FP8, tag="v8")
                for c in range(N_CHUNKS):
                    psv = ps_s.tile([128, 128], FP32, tag="sc")
                    nc.tensor.matmul(
                        psv, xgT[:, c * 128 : (c + 1) * 128], w_bf["wv"],
                        start=True, stop=True,
                    )
                    nc.vector.tensor_copy(v8[:, c, :], psv)

                # ---------------- attention macros ----------------
                for m in range(N_MACROS):
                    tq0 = m * TQ_MACRO
                    ps_out = ps_o.tile([128, TQ_MACRO], FP32)
                    ps_den = ps_d.tile([128, TQ_MACRO], FP32)
                    prev = None  # deferred fp8 matmuls for chunk-pair j-1
                    for j in range(N_CPAIRS):
                        # scores for chunks 2j, 2j+1 (bf16), one ps tile each
                        sc_tiles = []
                        for ci in range(2):
                            c = 2 * j + ci
                            ksl = kt[:, c * 128 : (c + 1) * 128]
                            ps_sc = ps_s.tile([128, TQ_MACRO], FP32, tag="sc")
                            for h in range(TQ_MACRO // 512):
                                sl = slice(h * 512, (h + 1) * 512)
                                qsl = slice(tq0 + h * 512, tq0 + (h + 1) * 512)
                                nc.tensor.matmul(
                                    ps_sc[:, sl], ksl, qt[:, qsl],
                                    start=True, stop=True,
                                )
                            sc_tiles.append(ps_sc)
                        # PE: fp8 PV + den for the PREVIOUS pair (overlaps ACT j)
                        if prev is not None:
                            pj, ppt = prev
                            first, last = pj == 0, False
                            for h in range(TQ_MACRO // 512):
                                sl = slice(h * 512, (h + 1) * 512)
                                nc.tensor.matmul(
                                    ps_out[:, sl], v8[:, 2 * pj : 2 * pj + 2, :],
                                    ppt[:, :, sl], start=first, stop=last,
                                    perf_mode=DR,
                                )
                                nc.tensor.matmul(
                                    ps_den[:, sl], ones2_f8, ppt[:, :, sl],
                                    start=first, stop=last, perf_mode=DR,
                                )
                        # ScalarE: exp -> fp8 chunk-pair tile
                        pt2 = ptpool.tile([128, 2, TQ_MACRO], FP8)
                        for ci in range(2):
                            nc.scalar.activation(
                                pt2[:, ci, :], sc_tiles[ci],
                                mybir.ActivationFunctionType.Exp,
                                scale=INV_SCALE, bias=exp_bias,
                            )
                        prev = (j, pt2)
                    # drain last pair
                    pj, ppt = prev
                    for h in range(TQ_MACRO // 512):
                        sl = slice(h * 512, (h + 1) * 512)
                        nc.tensor.matmul(
                            ps_out[:, sl], v8[:, 2 * pj : 2 * pj + 2, :],
                            ppt[:, :, sl], start=False, stop=True, perf_mode=DR,
                        )
                        nc.tensor.matmul(
                            ps_den[:, sl], ones2_f8, ppt[:, :, sl],
                            start=False, stop=True, perf_mode=DR,
                        )

                    # ---------------- epilogue ----------------
                    recip = epi.tile([128, TQ_MACRO], FP32, tag="recip")
                    nc.vector.reciprocal(recip, ps_den)
                    onorm = epi.tile([128, TQ_MACRO], BF16, tag="onorm")
                    nc.vector.tensor_mul(onorm, ps_out, recip)
                    nc.vector.tensor_scalar_add(onorm, onorm, w_bf["bv"])
                    # transpose back to natural [tq, e] via PE (bf16)
                    onat = epi.tile([128, TQ_MACRO // 128, 128], FP32, tag="onat")
                    for j in range(TQ_MACRO // 128):
                        tp = ps_s.tile([128, 128], BF16, tag="sc")
                        nc.tensor.transpose(
                            tp, onorm[:, j * 128 : (j + 1) * 128], ident_bf
                        )
                        nc.vector.tensor_copy(onat[:, j, :], tp)
                    nc.gpsimd.dma_start(
                        out=outs[i][tq0 : tq0 + TQ_MACRO, :].rearrange(
                            "(c p) d -> p c d", p=128
                        ),
                        in_=onat,
                    )
    _split_multi_waits(nc)
    return nc


def _get_nc():
    global _NC_CACHE
    if _NC_CACHE is None:
        _NC_CACHE = build_nc()
    return _NC_CACHE


def kernel(**inputs: np.ndarray) -> np.ndarray:
    x = np.ascontiguousarray(inputs["x"], dtype=np.float32)
    Wq = np.asarray(inputs["Wq"], dtype=np.float32)
    Wk = np.asarray(inputs["Wk"], dtype=np.float32)
    Wv = np.asarray(inputs["Wv"], dtype=np.float32)
    bq = np.asarray(inputs["bq"], dtype=np.float32)
    bk = np.asarray(inputs["bk"], dtype=np.float32)
    bv = np.asarray(inputs["bv"], dtype=np.float32)

    nc = _get_nc()

    in_maps = []
    for core in range(N_CORES):
        m = {}
        for i in range(PAIRS_PER_CORE):
            pair = core * PAIRS_PER_CORE + i
            b, g = pair // G, pair % G
            sl = slice(g * GS, (g + 1) * GS)
            m[f"x{i}"] = np.ascontiguousarray(x[b, :, sl])
            m[f"wq{i}"] = np.ascontiguousarray(Wq[g])
            m[f"wk{i}"] = np.ascontiguousarray(Wk[g])
            m[f"wv{i}"] = np.ascontiguousarray(Wv[g])
            m[f"bq{i}"] = np.ascontiguousarray(bq[g].reshape(1, GS))
            m[f"bk{i}"] = np.ascontiguousarray(bk[g].reshape(1, GS))
            m[f"bv{i}"] = np.ascontiguousarray(bv[g].reshape(1, GS))
        in_maps.append(m)

    global _LAST_IN_MAPS
    _LAST_IN_MAPS = in_maps

    from concourse.bass_utils import run_bass_kernel_spmd

    res = run_bass_kernel_spmd(nc, in_maps, list(range(N_CORES)))

    y = np.empty((B, T, F), dtype=np.float32)
    for core in range(N_CORES):
        for i in range(PAIRS_PER_CORE):
            pair = core * PAIRS_PER_CORE + i
            b, g = pair // G, pair % G
            y[b, :, g * GS : (g + 1) * GS] = res.results[core][f"y{i}"]
    return y


# revision 10
# speedup vs baseline: 1.4784x; 1.0415x over previous
"""Grouped-Query Attention kernel for Trainium2 (8 NeuronCores, SPMD).

Problem: x [4, 4096, 512] fp32, per-group Dense Q/K/V (G=4 groups of 128
features), full softmax attention within each (batch, group) pair, output
re-concatenated to [4, 4096, 512].

Sharding: B*G = 16 fully independent attention problems -> 2 per core.
Per core, per pair, everything stays on-chip (SBUF 24MB).

Pipeline (v2):
  - x loaded fp32 (chunked DMA), cast bf16, PE-transposed to xgT [d, t]
  - Q^T/K^T = W^T xgT (bias folded) bf16; V natural [t, e] via
    xgT-chunk-stationary matmuls, quantized to fp8 e4m3
  - scores computed TRANSPOSED per 128-key chunk: S^T[ts, tq] = K_c Q^T
    (bf16 matmuls)
  - exp via ScalarE ACT with scale=1/sqrt(gs) and bias=-3 folded in
    (the e^-3 shift cancels in softmax; keeps exp < 240 = trn-e4m3 max),
    output DIRECTLY in fp8 e4m3 into chunk-pair tiles [128, 2, 1024]
  - P@V and the ones-denominator matmul run in fp8 DoubleRow perf mode
    (K=256: two key chunks per pass -> half the streamed columns)
  - epilogue: reciprocal, normalize (+bq... bv col), PE-transpose back
    (bf16), DMA out fp32
Compute dtype bf16 for scores, fp8 for probabilities/V (fp32 PSUM acc).
"""

import os
import sys

sys.path.insert(0, "/opt/trn_rl_repo")

import numpy as np

import concourse.bass as bass
import concourse.mybir as mybir
import concourse.tile as tile
from concourse.masks import make_identity

B, T, F, G = 4, 4096, 512, 4
GS = F // G  # 128
N_CORES = 8
PAIRS_PER_CORE = (B * G) // N_CORES  # 2
TQ_MACRO = 1024  # query tile width per softmax/psum round
N_MACROS = T // TQ_MACRO  # 4
N_CHUNKS = T // 128  # 32 key/time chunks
N_CPAIRS = N_CHUNKS // 2  # 16 chunk pairs (DoubleRow K=256)
INV_SCALE = float(1.0 / (np.sqrt(np.float32(GS)) + 1e-9))
EXP_SHIFT = -3.0  # exp(s - 3): cancels in softmax, keeps max < 240

FP32 = mybir.dt.float32
BF16 = mybir.dt.bfloat16
FP8 = mybir.dt.float8e4
DR = mybir.MatmulPerfMode.DoubleRow

_NC_CACHE = None
_LAST_IN_MAPS = None


def _split_multi_waits(nc):
    """Walrus codegen rejects instructions carrying more than one semaphore
    wait on several instruction structs (DMA DIRECT2D, tensor_scalar, LDW).
    Hoist all-but-the-last wait of any multi-wait instruction onto same-engine
    NoOps inserted immediately before it: the sequencer executes them in
    order, so the gating semantics are identical."""
    n_split = 0
    for func in nc.m.functions:
        for block in func.blocks:
            new = []
            for inst in block.instructions:
                si = inst.sync_info
                waits = list(si.on_wait) if (si is not None and si.on_wait) else []
                if len(waits) > 1:
                    for w in waits[:-1]:
                        nop = mybir.InstNoOp(
                            name=nc.get_next_instruction_name(), ins=[], outs=[]
                        )
                        nop.engine = inst.engine
                        nop.sync_info = mybir.SyncInfo(on_wait=[w], on_update=[])
                        new.append(nop)
                        n_split += 1
                    inst.sync_info = mybir.SyncInfo(
                        on_wait=[waits[-1]],
                        on_update=list(si.on_update) if si.on_update else [],
                    )
                new.append(inst)
            block.instructions = new
    return n_split


def build_nc():
    nc = bass.Bass()

    ins = []
    outs = []
    for i in range(PAIRS_PER_CORE):
        ins.append(
            dict(
                x=nc.declare_dram_parameter(f"x{i}", [T, GS], FP32, isOutput=False),
                wq=nc.declare_dram_parameter(f"wq{i}", [GS, GS], FP32, isOutput=False),
                wk=nc.declare_dram_parameter(f"wk{i}", [GS, GS], FP32, isOutput=False),
                wv=nc.declare_dram_parameter(f"wv{i}", [GS, GS], FP32, isOutput=False),
                bq=nc.declare_dram_parameter(f"bq{i}", [1, GS], FP32, isOutput=False),
                bk=nc.declare_dram_parameter(f"bk{i}", [1, GS], FP32, isOutput=False),
                bv=nc.declare_dram_parameter(f"bv{i}", [1, GS], FP32, isOutput=False),
            )
        )
        outs.append(nc.declare_dram_parameter(f"y{i}", [T, GS], FP32, isOutput=True))

    with tile.TileContext(nc) as tc:
        with (
            tc.tile_pool(name="consts", bufs=1) as consts,
            tc.tile_pool(name="bigsb", bufs=2) as bigsb,  # per-pair persistent
            tc.tile_pool(name="pt", bufs=3) as ptpool,  # exp'd prob chunk-pairs
            tc.tile_pool(name="epi", bufs=2) as epi,  # epilogue sbuf tiles
            tc.tile_pool(name="ps_s", bufs=2, space="PSUM") as ps_s,  # scores
            tc.tile_pool(name="ps_o", bufs=1, space="PSUM") as ps_o,  # out^T
            tc.tile_pool(name="ps_d", bufs=1, space="PSUM") as ps_d,  # denom
        ):
            ident_bf = consts.tile([128, 128], BF16)
            make_identity(nc, ident_bf)
            ones2_f8 = consts.tile([128, 2, 128], FP8)
            nc.vector.memset(ones2_f8, 1.0)
            exp_bias = consts.tile([128, 1], FP32)
            nc.vector.memset(exp_bias, EXP_SHIFT)

            # ---- prefetch: x + weights for BOTH pairs up-front ----
            xg_fs = []
            w_sb = []
            for i in range(PAIRS_PER_CORE):
                p = ins[i]
                xg_f = bigsb.tile([128, N_CHUNKS, 128], FP32, tag="xg_f")
                for d in range(8):
                    nc.sync.dma_start(
                        out=xg_f[:, d * 4 : (d + 1) * 4, :],
                        in_=p["x"][d * 512 : (d + 1) * 512, :].rearrange(
                            "(c p) d -> p c d", p=128
                        ),
                    )
                xg_fs.append(xg_f)
                wd = {}
                for nm in ("wq", "wk", "wv"):
                    wf = epi.tile([128, 128], FP32, tag=f"wf{nm}{i}")
                    nc.gpsimd.dma_start(out=wf, in_=p[nm][:, :])
                    wb = consts.tile([128, 128], BF16, tag=f"{nm}{i}")
                    nc.vector.tensor_copy(wb, wf)
                    wd[nm] = wb
                for nm in ("bq", "bk", "bv"):
                    bc = consts.tile([128, 1], FP32, tag=f"{nm}{i}")
                    nc.gpsimd.dma_start(
                        out=bc, in_=p[nm][:, :].rearrange("o d -> d o")
                    )
                    wd[nm] = bc
                w_sb.append(wd)

            pending_epi = []
            for i in range(PAIRS_PER_CORE):
                w_bf = w_sb[i]
                # ---------------- prologue ----------------
                xg_f = xg_fs[i]
                xg_b = bigsb.tile([128, N_CHUNKS, 128], BF16, tag="xg_b")
                for d in range(8):
                    nc.vector.tensor_copy(
                        xg_b[:, d * 4 : (d + 1) * 4, :], xg_f[:, d * 4 : (d + 1) * 4, :]
                    )

                # xgT [d, t] bf16 via PE transpose of 32 chunks
                xgT = bigsb.tile([128, T], BF16, tag="xgT")
                for c in range(N_CHUNKS):
                    pst = ps_s.tile([128, 128], BF16, tag="sc")
                    nc.tensor.transpose(pst, xg_b[:, c, :], ident_bf)
                    nc.vector.tensor_copy(xgT[:, c * 128 : (c + 1) * 128], pst)

                # Q^T/K^T [e, t] bf16 (bias added)
                qt = bigsb.tile([128, T], BF16, tag="qt")
                kt = bigsb.tile([128, T], BF16, tag="kt")
                for dst, wname, bname in ((qt, "wq", "bq"), (kt, "wk", "bk")):
                    for j in range(T // TQ_MACRO):
                        psq = ps_s.tile([128, TQ_MACRO], FP32, tag="sc")
                        for h in range(TQ_MACRO // 512):
                            sl = slice(h * 512, (h + 1) * 512)
                            tsl = slice(
                                j * TQ_MACRO + h * 512, j * TQ_MACRO + (h + 1) * 512
                            )
                            nc.tensor.matmul(
                                psq[:, sl], w_bf[wname], xgT[:, tsl],
                                start=True, stop=True,
                            )
                        dsl = slice(j * TQ_MACRO, (j + 1) * TQ_MACRO)
                        nc.vector.tensor_scalar_add(dst[:, dsl], psq, w_bf[bname])

                # V natural [t, e] fp8 via xgT-chunk-stationary matmuls
                v8 = bigsb.tile([128, N_CHUNKS, 128], FP8, tag="v8")
                for c in range(N_CHUNKS):
                    psv = ps_s.tile([128, 128], FP32, tag="sc")
                    nc.tensor.matmul(
                        psv, xgT[:, c * 128 : (c + 1) * 128], w_bf["wv"],
                        start=True, stop=True,
                    )
                    nc.vector.tensor_copy(v8[:, c, :], psv)

                # ---------------- attention macros ----------------
                for m in range(N_MACROS):
                    tq0 = m * TQ_MACRO
                    ps_out = ps_o.tile([128, TQ_MACRO], FP32)
                    ps_den = ps_d.tile([128, TQ_MACRO], FP32)
                    prev = None  # deferred fp8 matmuls for chunk-pair j-1
                    for j in range(N_CPAIRS):
                        if j == 2 and pending_epi:
                            pending_epi.pop(0)()
                        # scores for chunks 2j, 2j+1 (bf16), one ps tile each
                        sc_tiles = []
                        for ci in range(2):
                            c = 2 * j + ci
                            ksl = kt[:, c * 128 : (c + 1) * 128]
                            ps_sc = ps_s.tile([128, TQ_MACRO], FP32, tag="sc")
                            for h in range(TQ_MACRO // 512):
                                sl = slice(h * 512, (h + 1) * 512)
                                qsl = slice(tq0 + h * 512, tq0 + (h + 1) * 512)
                                nc.tensor.matmul(
                                    ps_sc[:, sl], ksl, qt[:, qsl],
                                    start=True, stop=True,
                                )
                            sc_tiles.append(ps_sc)
                        # PE: fp8 PV + den for the PREVIOUS pair (overlaps ACT j)
                        if prev is not None:
                            pj, ppt = prev
                            first, last = pj == 0, False
                            for h in range(TQ_MACRO // 512):
                                sl = slice(h * 512, (h + 1) * 512)
                                nc.tensor.matmul(
                                    ps_out[:, sl], v8[:, 2 * pj : 2 * pj + 2, :],
                                    ppt[:, :, sl], start=first, stop=last,
                                    perf_mode=DR,
                                )
                                nc.tensor.matmul(
                                    ps_den[:, sl], ones2_f8, ppt[:, :, sl],
                                    start=first, stop=last, perf_mode=DR,
                                )
                        # ScalarE: exp -> fp8 chunk-pair tile
                        pt2 = ptpool.tile([128, 2, TQ_MACRO], FP8)
                        for ci in range(2):
                            nc.scalar.activation(
                                pt2[:, ci, :], sc_tiles[ci],
                                mybir.ActivationFunctionType.Exp,
                                scale=INV_SCALE, bias=exp_bias,
                            )
                        prev = (j, pt2)
                    # drain last pair
                    pj, ppt = prev
                    for h in range(TQ_MACRO // 512):
                        sl = slice(h * 512, (h + 1) * 512)
                        nc.tensor.matmul(
                            ps_out[:, sl], v8[:, 2 * pj : 2 * pj + 2, :],
                            ppt[:, :, sl], start=False, stop=True, perf_mode=DR,
                        )
                        nc.tensor.matmul(
                            ps_den[:, sl], ones2_f8, ppt[:, :, sl],
                            start=False, stop=True, perf_mode=DR,
                        )

                    # ---------------- epilogue ----------------
                    # DVE normalization now; PE transposes deferred into the
                    # next macro's chunk loop so PE never idles here.
                    recip = epi.tile([128, TQ_MACRO], FP32, tag="recip")
                    nc.vector.reciprocal(recip, ps_den)
                    onorm = epi.tile([128, TQ_MACRO], BF16, tag="onorm")
                    nc.vector.tensor_mul(onorm, ps_out, recip)
                    nc.vector.tensor_scalar_add(onorm, onorm, w_bf["bv"])

                    def _epi(onorm=onorm, tq0=tq0, out_dram=outs[i]):
                        onat = epi.tile(
                            [128, TQ_MACRO // 128, 128], FP32, tag="onat"
                        )
                        for j in range(TQ_MACRO // 128):
                            tp = ps_s.tile([128, 128], BF16, tag="sc")
                            nc.tensor.transpose(
                                tp, onorm[:, j * 128 : (j + 1) * 128], ident_bf
                            )
                            nc.vector.tensor_copy(onat[:, j, :], tp)
                            if j % 4 == 3:
                                h = j // 4
                                nc.gpsimd.dma_start(
                                    out=out_dram[
                                        tq0 + h * 512 : tq0 + (h + 1) * 512, :
                                    ].rearrange("(c p) d -> p c d", p=128),
                                    in_=onat[:, h * 4 : (h + 1) * 4, :],
                                )

                    pending_epi.append(_epi)
            for f in pending_epi:
                f()
    _split_multi_waits(nc)
    return nc


def _get_nc():
    global _NC_CACHE
    if _NC_CACHE is None:
        _NC_CACHE = build_nc()
    return _NC_CACHE


def kernel(**inputs: np.ndarray) -> np.ndarray:
    x = np.ascontiguousarray(inputs["x"], dtype=np.float32)
    Wq = np.asarray(inputs["Wq"], dtype=np.float32)
    Wk = np.asarray(inputs["Wk"], dtype=np.float32)
    Wv = np.asarray(inputs["Wv"], dtype=np.float32)
    bq = np.asarray(inputs["bq"], dtype=np.float32)
    bk = np.asarray(inputs["bk"], dtype=np.float32)
    bv = np.asarray(inputs["bv"], dtype=np.float32)

    nc = _get_nc()

    in_maps = []
    for core in range(N_CORES):
        m = {}
        for i in range(PAIRS_PER_CORE):
            pair = core * PAIRS_PER_CORE + i
            b, g = pair // G, pair % G
            sl = slice(g * GS, (g + 1) * GS)
            m[f"x{i}"] = np.ascontiguousarray(x[b, :, sl])
            m[f"wq{i}"] = np.ascontiguousarray(Wq[g])
            m[f"wk{i}"] = np.ascontiguousarray(Wk[g])
            m[f"wv{i}"] = np.ascontiguousarray(Wv[g])
            m[f"bq{i}"] = np.ascontiguousarray(bq[g].reshape(1, GS))
            m[f"bk{i}"] = np.ascontiguousarray(bk[g].reshape(1, GS))
            m[f"bv{i}"] = np.ascontiguousarray(bv[g].reshape(1, GS))
        in_maps.append(m)

    global _LAST_IN_MAPS
    _LAST_IN_MAPS = in_maps

    from concourse.bass_utils import run_bass_kernel_spmd

    res = run_bass_kernel_spmd(nc, in_maps, list(range(N_CORES)))

    y = np.empty((B, T, F), dtype=np.float32)
    for core in range(N_CORES):
        for i in range(PAIRS_PER_CORE):
            pair = core * PAIRS_PER_CORE + i
            b, g = pair // G, pair % G
            y[b, :, g * GS : (g + 1) * GS] = res.results[core][f"y{i}"]
    return y


# revision 13
# speedup vs baseline: 1.4897x; 1.0076x over previous
"""Grouped-Query Attention kernel for Trainium2 (8 NeuronCores, SPMD).

Problem: x [4, 4096, 512] fp32, per-group Dense Q/K/V (G=4 groups of 128
features), full softmax attention within each (batch, group) pair, output
re-concatenated to [4, 4096, 512].

Sharding: B*G = 16 fully independent attention problems -> 2 per core.
Per core, per pair, everything stays on-chip (SBUF 24MB).

Pipeline (v2):
  - x loaded fp32 (chunked DMA), cast bf16, PE-transposed to xgT [d, t]
  - Q^T/K^T = W^T xgT (bias folded) bf16; V natural [t, e] via
    xgT-chunk-stationary matmuls, quantized to fp8 e4m3
  - scores computed TRANSPOSED per 128-key chunk: S^T[ts, tq] = K_c Q^T
    (bf16 matmuls)
  - exp via ScalarE ACT with scale=1/sqrt(gs) and bias=-3 folded in
    (the e^-3 shift cancels in softmax; keeps exp < 240 = trn-e4m3 max),
    output DIRECTLY in fp8 e4m3 into chunk-pair tiles [128, 2, 1024]
  - P@V and the ones-denominator matmul run in fp8 DoubleRow perf mode
    (K=256: two key chunks per pass -> half the streamed columns)
  - epilogue: reciprocal, normalize (+bq... bv col), PE-transpose back
    (bf16), DMA out fp32
Compute dtype bf16 for scores, fp8 for probabilities/V (fp32 PSUM acc).
"""

import os
import sys

sys.path.insert(0, "/opt/trn_rl_repo")

import numpy as np

import concourse.bass as bass
import concourse.mybir as mybir
import concourse.tile as tile
from concourse.masks import make_identity

B, T, F, G = 4, 4096, 512, 4
GS = F // G  # 128
N_CORES = 8
PAIRS_PER_CORE = (B * G) // N_CORES  # 2
TQ_MACRO = 1024  # query tile width per softmax/psum round
N_MACROS = T // TQ_MACRO  # 4
N_CHUNKS = T // 128  # 32 key/time chunks
N_CPAIRS = N_CHUNKS // 2  # 16 chunk pairs (DoubleRow K=256)
INV_SCALE = float(1.0 / (np.sqrt(np.float32(GS)) + 1e-9))
EXP_SHIFT = -3.0  # exp(s - 3): cancels in softmax, keeps max < 240

FP32 = mybir.dt.float32
BF16 = mybir.dt.bfloat16
FP8 = mybir.dt.float8e4
DR = mybir.MatmulPerfMode.DoubleRow

_NC_CACHE = None
_LAST_IN_MAPS = None


def _split_multi_waits(nc):
    """Walrus codegen rejects instructions carrying more than one semaphore
    wait on several instruction structs (DMA DIRECT2D, tensor_scalar, LDW).
    Hoist all-but-the-last wait of any multi-wait instruction onto same-engine
    NoOps inserted immediately before it: the sequencer executes them in
    order, so the gating semantics are identical."""
    n_split = 0
    for func in nc.m.functions:
        for block in func.blocks:
            new = []
            for inst in block.instructions:
                si = inst.sync_info
                waits = list(si.on_wait) if (si is not None and si.on_wait) else []
                if len(waits) > 1:
                    for w in waits[:-1]:
                        nop = mybir.InstNoOp(
                            name=nc.get_next_instruction_name(), ins=[], outs=[]
                        )
                        nop.engine = inst.engine
                        nop.sync_info = mybir.SyncInfo(on_wait=[w], on_update=[])
                        new.append(nop)
                        n_split += 1
                    inst.sync_info = mybir.SyncInfo(
                        on_wait=[waits[-1]],
                        on_update=list(si.on_update) if si.on_update else [],
                    )
                new.append(inst)
            block.instructions = new
    return n_split


def build_nc():
    nc = bass.Bass()

    ins = []
    outs = []
    for i in range(PAIRS_PER_CORE):
        ins.append(
            dict(
                x=nc.declare_dram_parameter(f"x{i}", [T, GS], FP32, isOutput=False),
                wq=nc.declare_dram_parameter(f"wq{i}", [GS, GS], FP32, isOutput=False),
                wk=nc.declare_dram_parameter(f"wk{i}", [GS, GS], FP32, isOutput=False),
                wv=nc.declare_dram_parameter(f"wv{i}", [GS, GS], FP32, isOutput=False),
                bq=nc.declare_dram_parameter(f"bq{i}", [1, GS], FP32, isOutput=False),
                bk=nc.declare_dram_parameter(f"bk{i}", [1, GS], FP32, isOutput=False),
                bv=nc.declare_dram_parameter(f"bv{i}", [1, GS], FP32, isOutput=False),
            )
        )
        outs.append(nc.declare_dram_parameter(f"y{i}", [T, GS], FP32, isOutput=True))

    with tile.TileContext(nc) as tc:
        with (
            tc.tile_pool(name="consts", bufs=1) as consts,
            tc.tile_pool(name="bigsb", bufs=2) as bigsb,  # per-pair persistent
            tc.tile_pool(name="pt", bufs=4) as ptpool,  # exp'd prob chunk-pairs
            tc.tile_pool(name="epi", bufs=2) as epi,  # epilogue sbuf tiles
            tc.tile_pool(name="ps_s", bufs=2, space="PSUM") as ps_s,  # scores
            tc.tile_pool(name="ps_o", bufs=1, space="PSUM") as ps_o,  # out^T
            tc.tile_pool(name="ps_d", bufs=1, space="PSUM") as ps_d,  # denom
        ):
            ident_bf = consts.tile([128, 128], BF16)
            make_identity(nc, ident_bf)
            ones2_f8 = consts.tile([128, 2, 128], FP8)
            nc.vector.memset(ones2_f8, 1.0)
            exp_bias = consts.tile([128, 1], FP32)
            nc.vector.memset(exp_bias, EXP_SHIFT)

            # ---- prefetch: x + weights for BOTH pairs up-front ----
            xg_fs = []
            w_sb = []
            for i in range(PAIRS_PER_CORE):
                p = ins[i]
                xg_f = bigsb.tile([128, N_CHUNKS, 128], FP32, tag="xg_f")
                for d in range(8):
                    nc.sync.dma_start(
                        out=xg_f[:, d * 4 : (d + 1) * 4, :],
                        in_=p["x"][d * 512 : (d + 1) * 512, :].rearrange(
                            "(c p) d -> p c d", p=128
                        ),
                    )
                xg_fs.append(xg_f)
                wd = {}
                for nm in ("wq", "wk", "wv"):
                    wf = epi.tile([128, 128], FP32, tag=f"wf{nm}{i}")
                    nc.gpsimd.dma_start(out=wf, in_=p[nm][:, :])
                    wb = consts.tile([128, 128], BF16, tag=f"{nm}{i}")
                    nc.vector.tensor_copy(wb, wf)
                    wd[nm] = wb
                for nm in ("bq", "bk", "bv"):
                    bc = consts.tile([128, 1], FP32, tag=f"{nm}{i}")
                    nc.gpsimd.dma_start(
                        out=bc, in_=p[nm][:, :].rearrange("o d -> d o")
                    )
                    wd[nm] = bc
                w_sb.append(wd)

            pending_epi = []
            pv_queue = []
            for i in range(PAIRS_PER_CORE):
                w_bf = w_sb[i]
                # ---------------- prologue ----------------
                xg_f = xg_fs[i]
                xg_b = bigsb.tile([128, N_CHUNKS, 128], BF16, tag="xg_b")
                for d in range(8):
                    nc.vector.tensor_copy(
                        xg_b[:, d * 4 : (d + 1) * 4, :], xg_f[:, d * 4 : (d + 1) * 4, :]
                    )

                # xgT [d, t] bf16 via PE transpose of 32 chunks
                xgT = bigsb.tile([128, T], BF16, tag="xgT")
                for c in range(N_CHUNKS):
                    pst = ps_s.tile([128, 128], BF16, tag="sc")
                    nc.tensor.transpose(pst, xg_b[:, c, :], ident_bf)
                    nc.vector.tensor_copy(xgT[:, c * 128 : (c + 1) * 128], pst)

                # Q^T/K^T [e, t] bf16 (bias added)
                qt = bigsb.tile([128, T], BF16, tag="qt")
                kt = bigsb.tile([128, T], BF16, tag="kt")
                for dst, wname, bname in ((qt, "wq", "bq"), (kt, "wk", "bk")):
                    for j in range(T // TQ_MACRO):
                        psq = ps_s.tile([128, TQ_MACRO], FP32, tag="sc")
                        for h in range(TQ_MACRO // 512):
                            sl = slice(h * 512, (h + 1) * 512)
                            tsl = slice(
                                j * TQ_MACRO + h * 512, j * TQ_MACRO + (h + 1) * 512
                            )
                            nc.tensor.matmul(
                                psq[:, sl], w_bf[wname], xgT[:, tsl],
                                start=True, stop=True,
                            )
                        dsl = slice(j * TQ_MACRO, (j + 1) * TQ_MACRO)
                        nc.vector.tensor_scalar_add(dst[:, dsl], psq, w_bf[bname])

                # V natural [t, e] fp8 via xgT-chunk-stationary matmuls
                v8 = bigsb.tile([128, N_CHUNKS, 128], FP8, tag="v8")
                for c in range(N_CHUNKS):
                    psv = ps_s.tile([128, 128], FP32, tag="sc")
                    nc.tensor.matmul(
                        psv, xgT[:, c * 128 : (c + 1) * 128], w_bf["wv"],
                        start=True, stop=True,
                    )
                    nc.vector.tensor_copy(v8[:, c, :], psv)

                # ---------------- attention macros ----------------
                # Cross-macro software pipeline: the fp8 PV+den matmuls for
                # chunk-pair j are emitted 2 iterations later (possibly in the
                # NEXT macro), so the PE never waits on the trailing ACTs and
                # HAM stays warm. The last PV unit of a macro also emits that
                # macro's DVE normalization and queues the PE transposes.
                for m in range(N_MACROS):
                    tq0 = m * TQ_MACRO
                    ps_out = ps_o.tile([128, TQ_MACRO], FP32)
                    ps_den = ps_d.tile([128, TQ_MACRO], FP32)

                    def _mk_pv(pj, ppt, ps_out=ps_out, ps_den=ps_den,
                               tq0=tq0, w_bf=w_bf, v8=v8, out_dram=outs[i]):
                        def _pv():
                            first, last = pj == 0, pj == N_CPAIRS - 1
                            for h in range(TQ_MACRO // 512):
                                sl = slice(h * 512, (h + 1) * 512)
                                nc.tensor.matmul(
                                    ps_out[:, sl], v8[:, 2 * pj : 2 * pj + 2, :],
                                    ppt[:, :, sl], start=first, stop=last,
                                    perf_mode=DR,
                                )
                                nc.tensor.matmul(
                                    ps_den[:, sl], ones2_f8, ppt[:, :, sl],
                                    start=first, stop=last, perf_mode=DR,
                                )
                            if not last:
                                return
                            # macro complete: DVE normalization + queue PE
                            # transposes for a later slot
                            recip = epi.tile([128, TQ_MACRO], FP32, tag="recip")
                            nc.vector.reciprocal(recip, ps_den)
                            onorm = epi.tile([128, TQ_MACRO], BF16, tag="onorm")
                            nc.vector.tensor_mul(onorm, ps_out, recip)
                            nc.vector.tensor_scalar_add(onorm, onorm, w_bf["bv"])

                            def _epi():
                                onat = epi.tile(
                                    [128, TQ_MACRO // 128, 128], FP32, tag="onat"
                                )
                                for jj in range(TQ_MACRO // 128):
                                    tp = ps_s.tile([128, 128], BF16, tag="sc")
                                    nc.tensor.transpose(
                                        tp,
                                        onorm[:, jj * 128 : (jj + 1) * 128],
                                        ident_bf,
                                    )
                                    nc.vector.tensor_copy(onat[:, jj, :], tp)
                                    if jj % 4 == 3:
                                        hh = jj // 4
                                        nc.gpsimd.dma_start(
                                            out=out_dram[
                                                tq0 + hh * 512 : tq0 + (hh + 1) * 512,
                                                :,
                                            ].rearrange("(c p) d -> p c d", p=128),
                                            in_=onat[:, hh * 4 : (hh + 1) * 4, :],
                                        )

                            pending_epi.append(_epi)

                        return _pv

                    for j in range(N_CPAIRS):
                        # scores for chunks 2j, 2j+1 (bf16), one ps tile each
                        sc_tiles = []
                        for ci in range(2):
                            c = 2 * j + ci
                            ksl = kt[:, c * 128 : (c + 1) * 128]
                            ps_sc = ps_s.tile([128, TQ_MACRO], FP32, tag="sc")
                            for h in range(TQ_MACRO // 512):
                                sl = slice(h * 512, (h + 1) * 512)
                                qsl = slice(tq0 + h * 512, tq0 + (h + 1) * 512)
                                nc.tensor.matmul(
                                    ps_sc[:, sl], ksl, qt[:, qsl],
                                    start=True, stop=True,
                                )
                            sc_tiles.append(ps_sc)
                        if j == 2 and pending_epi:
                            pending_epi.pop(0)()
                        # PE: fp8 PV + den, 2 chunk-pairs behind
                        while len(pv_queue) > 1:
                            pv_queue.pop(0)()
                        # ScalarE: exp -> fp8 chunk-pair tile
                        pt2 = ptpool.tile([128, 2, TQ_MACRO], FP8)
                        for ci in range(2):
                            nc.scalar.activation(
                                pt2[:, ci, :], sc_tiles[ci],
                                mybir.ActivationFunctionType.Exp,
                                scale=INV_SCALE, bias=exp_bias,
                            )
                        pv_queue.append(_mk_pv(j, pt2))
            while pv_queue:
                pv_queue.pop(0)()
            for f in pending_epi:
                f()
    _split_multi_waits(nc)
    return nc


def _get_nc():
    global _NC_CACHE
    if _NC_CACHE is None:
        _NC_CACHE = build_nc()
    return _NC_CACHE


def kernel(**inputs: np.ndarray) -> np.ndarray:
    x = np.ascontiguousarray(inputs["x"], dtype=np.float32)
    Wq = np.asarray(inputs["Wq"], dtype=np.float32)
    Wk = np.asarray(inputs["Wk"], dtype=np.float32)
    Wv = np.asarray(inputs["Wv"], dtype=np.float32)
    bq = np.asarray(inputs["bq"], dtype=np.float32)
    bk = np.asarray(inputs["bk"], dtype=np.float32)
    bv = np.asarray(inputs["bv"], dtype=np.float32)

    nc = _get_nc()

    in_maps = []
    for core in range(N_CORES):
        m = {}
        for i in range(PAIRS_PER_CORE):
            pair = core * PAIRS_PER_CORE + i
            b, g = pair // G, pair % G
            sl = slice(g * GS, (g + 1) * GS)
            m[f"x{i}"] = np.ascontiguousarray(x[b, :, sl])
            m[f"wq{i}"] = np.ascontiguousarray(Wq[g])
            m[f"wk{i}"] = np.ascontiguousarray(Wk[g])
            m[f"wv{i}"] = np.ascontiguousarray(Wv[g])
            m[f"bq{i}"] = np.ascontiguousarray(bq[g].reshape(1, GS))
            m[f"bk{i}"] = np.ascontiguousarray(bk[g].reshape(1, GS))
            m[f"bv{i}"] = np.ascontiguousarray(bv[g].reshape(1, GS))
        in_maps.append(m)

    global _LAST_IN_MAPS
    _LAST_IN_MAPS = in_maps

    from concourse.bass_utils import run_bass_kernel_spmd

    res = run_bass_kernel_spmd(nc, in_maps, list(range(N_CORES)))

    y = np.empty((B, T, F), dtype=np.float32)
    for core in range(N_CORES):
        for i in range(PAIRS_PER_CORE):
            pair = core * PAIRS_PER_CORE + i
            b, g = pair // G, pair % G
            y[b, :, g * GS : (g + 1) * GS] = res.results[core][f"y{i}"]
    return y


# revision 15
# speedup vs baseline: 1.6602x; 1.1145x over previous
"""Grouped-Query Attention kernel for Trainium2 (8 NeuronCores, SPMD).

Problem: x [4, 4096, 512] fp32, per-group Dense Q/K/V (G=4 groups of 128
features), full softmax attention within each (batch, group) pair, output
re-concatenated to [4, 4096, 512].

Sharding: B*G = 16 fully independent attention problems -> 2 per core.
Per core, per pair, everything stays on-chip (SBUF 24MB).

Pipeline (v2):
  - x loaded fp32 (chunked DMA), cast bf16, PE-transposed to xgT [d, t]
  - Q^T/K^T = W^T xgT (bias folded) bf16; V natural [t, e] via
    xgT-chunk-stationary matmuls, quantized to fp8 e4m3
  - scores computed TRANSPOSED per 128-key chunk: S^T[ts, tq] = K_c Q^T
    (bf16 matmuls)
  - exp via ScalarE ACT with scale=1/sqrt(gs) and bias=-3 folded in
    (the e^-3 shift cancels in softmax; keeps exp < 240 = trn-e4m3 max),
    output DIRECTLY in fp8 e4m3 into chunk-pair tiles [128, 2, 1024]
  - P@V and the ones-denominator matmul run in fp8 DoubleRow perf mode
    (K=256: two key chunks per pass -> half the streamed columns)
  - epilogue: reciprocal, normalize (+bq... bv col), PE-transpose back
    (bf16), DMA out fp32
Compute dtype bf16 for scores, fp8 for probabilities/V (fp32 PSUM acc).
"""

import os
import sys

sys.path.insert(0, "/opt/trn_rl_repo")

import numpy as np

import concourse.bass as bass
import concourse.mybir as mybir
import concourse.tile as tile
from concourse.masks import make_identity

B, T, F, G = 4, 4096, 512, 4
GS = F // G  # 128
N_CORES = 8
PAIRS_PER_CORE = (B * G) // N_CORES  # 2
TQ_MACRO = 1024  # query tile width per softmax/psum round
N_MACROS = T // TQ_MACRO  # 4
N_CHUNKS = T // 128  # 32 key/time chunks
N_CPAIRS = N_CHUNKS // 2  # 16 chunk pairs (DoubleRow K=256)
INV_SCALE = float(1.0 / (np.sqrt(np.float32(GS)) + 1e-9))
EXP_SHIFT = -3.0  # exp(s - 3): cancels in softmax, keeps max < 240

FP32 = mybir.dt.float32
BF16 = mybir.dt.bfloat16
FP8 = mybir.dt.float8e4
DR = mybir.MatmulPerfMode.DoubleRow

_NC_CACHE = None
_LAST_IN_MAPS = None


def _split_multi_waits(nc):
    """Walrus codegen rejects instructions carrying more than one semaphore
    wait on several instruction structs (DMA DIRECT2D, tensor_scalar, LDW).
    Hoist all-but-the-last wait of any multi-wait instruction onto same-engine
    NoOps inserted immediately before it: the sequencer executes them in
    order, so the gating semantics are identical."""
    n_split = 0
    for func in nc.m.functions:
        for block in func.blocks:
            new = []
            for inst in block.instructions:
                si = inst.sync_info
                waits = list(si.on_wait) if (si is not None and si.on_wait) else []
                if len(waits) > 1:
                    for w in waits[:-1]:
                        nop = mybir.InstNoOp(
                            name=nc.get_next_instruction_name(), ins=[], outs=[]
                        )
                        nop.engine = inst.engine
                        nop.sync_info = mybir.SyncInfo(on_wait=[w], on_update=[])
                        new.append(nop)
                        n_split += 1
                    inst.sync_info = mybir.SyncInfo(
                        on_wait=[waits[-1]],
                        on_update=list(si.on_update) if si.on_update else [],
                    )
                new.append(inst)
            block.instructions = new
    return n_split


def build_nc():
    nc = bass.Bass()

    ins = []
    outs = []
    for i in range(PAIRS_PER_CORE):
        ins.append(
            dict(
                x=nc.declare_dram_parameter(f"x{i}", [T, GS], FP32, isOutput=False),
                wq=nc.declare_dram_parameter(f"wq{i}", [GS, GS], FP32, isOutput=False),
                wk=nc.declare_dram_parameter(f"wk{i}", [GS, GS], FP32, isOutput=False),
                wv=nc.declare_dram_parameter(f"wv{i}", [GS, GS], FP32, isOutput=False),
                bq=nc.declare_dram_parameter(f"bq{i}", [1, GS], FP32, isOutput=False),
                bk=nc.declare_dram_parameter(f"bk{i}", [1, GS], FP32, isOutput=False),
                bv=nc.declare_dram_parameter(f"bv{i}", [1, GS], FP32, isOutput=False),
            )
        )
        outs.append(nc.declare_dram_parameter(f"y{i}", [T, GS], FP32, isOutput=True))

    with tile.TileContext(nc) as tc:
        with (
            tc.tile_pool(name="consts", bufs=1) as consts,
            tc.tile_pool(name="bigsb", bufs=2) as bigsb,  # per-pair persistent
            tc.tile_pool(name="pt", bufs=4) as ptpool,  # exp'd prob chunk-pairs
            tc.tile_pool(name="epi", bufs=2) as epi,  # epilogue sbuf tiles
            tc.tile_pool(name="ps_s", bufs=2, space="PSUM") as ps_s,  # scores
            tc.tile_pool(name="ps_o", bufs=1, space="PSUM") as ps_o,  # out^T
            tc.tile_pool(name="ps_d", bufs=1, space="PSUM") as ps_d,  # denom
        ):
            ident_bf = consts.tile([128, 128], BF16)
            make_identity(nc, ident_bf)
            ones2_f8 = consts.tile([128, 2, 128], FP8)
            nc.vector.memset(ones2_f8, 1.0)
            exp_bias = consts.tile([128, 1], FP32)
            nc.vector.memset(exp_bias, EXP_SHIFT)

            # ---- prefetch: x + weights for BOTH pairs up-front ----
            xg_fs = []
            w_sb = []
            for i in range(PAIRS_PER_CORE):
                p = ins[i]
                xg_f = bigsb.tile([128, N_CHUNKS, 128], FP32, tag="xg_f")
                for d in range(8):
                    nc.sync.dma_start(
                        out=xg_f[:, d * 4 : (d + 1) * 4, :],
                        in_=p["x"][d * 512 : (d + 1) * 512, :].rearrange(
                            "(c p) d -> p c d", p=128
                        ),
                    )
                xg_fs.append(xg_f)
                wd = {}
                for nm in ("wq", "wk", "wv"):
                    wf = epi.tile([128, 128], FP32, tag=f"wf{nm}{i}")
                    nc.gpsimd.dma_start(out=wf, in_=p[nm][:, :])
                    wb = consts.tile([128, 128], BF16, tag=f"{nm}{i}")
                    nc.vector.tensor_copy(wb, wf)
                    wd[nm] = wb
                for nm in ("bq", "bk"):
                    bc = consts.tile([128, 1], FP32, tag=f"{nm}{i}")
                    nc.gpsimd.dma_start(
                        out=bc, in_=p[nm][:, :].rearrange("o d -> d o")
                    )
                    wd[nm] = bc
                # bv broadcast across partitions [128, 128] (folded into V)
                bvb = consts.tile([128, 128], FP32, tag=f"bvb{i}")
                _bv = p["bv"][:, :]
                nc.gpsimd.dma_start(
                    out=bvb,
                    in_=bass.AP(tensor=_bv.tensor, offset=_bv.offset,
                                ap=[[0, 128]] + list(_bv.ap[1:])),
                )
                wd["bvb"] = bvb
                w_sb.append(wd)

            # per-pair persistent tiles (bigsb bufs=2 rotates by tag)
            pair_tiles = []
            for i in range(PAIRS_PER_CORE):
                pair_tiles.append(dict(
                    xg_b=bigsb.tile([128, N_CHUNKS, 128], BF16, tag="xg_b",
                                    name=f"xg_b{i}"),
                    xgT=bigsb.tile([128, T], BF16, tag="xgT", name=f"xgT{i}"),
                    qt=bigsb.tile([128, T], BF16, tag="qt", name=f"qt{i}"),
                    kt=bigsb.tile([128, T], BF16, tag="kt", name=f"kt{i}"),
                    v8=bigsb.tile([128, N_CHUNKS, 128], FP8, tag="v8",
                                  name=f"v8_{i}"),
                ))

            def prologue_gen(i):
                """Yields between op batches (~0.5us PE each) so the caller
                can interleave this pair's prologue into the previous pair's
                macro loop."""
                w_bf = w_sb[i]
                pt = pair_tiles[i]
                xg_f, xg_b, xgT = xg_fs[i], pt["xg_b"], pt["xgT"]
                for d in range(8):
                    nc.vector.tensor_copy(
                        xg_b[:, d * 4 : (d + 1) * 4, :],
                        xg_f[:, d * 4 : (d + 1) * 4, :],
                    )
                # xgT [d, t] bf16 via PE transpose of 32 chunks
                for c in range(N_CHUNKS):
                    pst = ps_s.tile([128, 128], BF16, tag="sc")
                    nc.tensor.transpose(pst, xg_b[:, c, :], ident_bf)
                    nc.vector.tensor_copy(xgT[:, c * 128 : (c + 1) * 128], pst)
                    if c % 2 == 1:
                        yield
                # Q^T/K^T [e, t] bf16 (bias added)
                for dst, wname, bname in (
                    (pt["qt"], "wq", "bq"), (pt["kt"], "wk", "bk")
                ):
                    for j in range(T // TQ_MACRO):
                        psq = ps_s.tile([128, TQ_MACRO], FP32, tag="sc")
                        for h in range(TQ_MACRO // 512):
                            sl = slice(h * 512, (h + 1) * 512)
                            tsl = slice(
                                j * TQ_MACRO + h * 512, j * TQ_MACRO + (h + 1) * 512
                            )
                            nc.tensor.matmul(
                                psq[:, sl], w_bf[wname], xgT[:, tsl],
                                start=True, stop=True,
                            )
                        dsl = slice(j * TQ_MACRO, (j + 1) * TQ_MACRO)
                        nc.vector.tensor_scalar_add(dst[:, dsl], psq, w_bf[bname])
                        yield
                # V natural [t, e] fp8 (+bv folded in)
                for c in range(N_CHUNKS):
                    psv = ps_s.tile([128, 128], FP32, tag="sc")
                    nc.tensor.matmul(
                        psv, xgT[:, c * 128 : (c + 1) * 128], w_bf["wv"],
                        start=True, stop=True,
                    )
                    nc.vector.tensor_add(pt["v8"][:, c, :], psv, w_bf["bvb"])
                    if c % 2 == 1:
                        yield

            pending_epi = []
            pv_queue = []
            pro = [prologue_gen(i) for i in range(PAIRS_PER_CORE)]

            def pump(gen_idx, n=1):
                if gen_idx >= PAIRS_PER_CORE:
                    return
                g = pro[gen_idx]
                if g is None:
                    return
                try:
                    for _ in range(n):
                        next(g)
                except StopIteration:
                    pro[gen_idx] = None

            pump(0, 10**9)  # pair 0 prologue can't hide behind anything

            for i in range(PAIRS_PER_CORE):
                pt_i = pair_tiles[i]
                qt, kt, v8 = pt_i["qt"], pt_i["kt"], pt_i["v8"]
                pump(i, 10**9)  # finish any prologue remainder

                # ---------------- attention macros ----------------
                # Cross-macro software pipeline: the fp8 PV+den matmuls for
                # chunk-pair j are emitted 2 iterations later (possibly in the
                # NEXT macro), so the PE never waits on the trailing ACTs and
                # HAM stays warm. The last PV unit of a macro also frees PSUM
                # via cheap bf16 copies and queues the PE transpose epilogue.
                for m in range(N_MACROS):
                    tq0 = m * TQ_MACRO
                    ps_out = ps_o.tile([128, TQ_MACRO], FP32)
                    ps_den = ps_d.tile([128, TQ_MACRO], FP32)

                    def _mk_pv(pj, ppt, ps_out=ps_out, ps_den=ps_den,
                               tq0=tq0, v8=v8, out_dram=outs[i]):
                        def _pv():
                            first, last = pj == 0, pj == N_CPAIRS - 1
                            for h in range(TQ_MACRO // 512):
                                sl = slice(h * 512, (h + 1) * 512)
                                nc.tensor.matmul(
                                    ps_out[:, sl], v8[:, 2 * pj : 2 * pj + 2, :],
                                    ppt[:, :, sl], start=first, stop=last,
                                    perf_mode=DR,
                                )
                                nc.tensor.matmul(
                                    ps_den[:, sl], ones2_f8, ppt[:, :, sl],
                                    start=first, stop=last, perf_mode=DR,
                                )
                            if not last:
                                return
                            # macro complete: evacuate PSUM via fast bf16
                            # copies (frees ps_out/ps_den for the next macro),
                            # defer transposes + normalization.
                            ot_b = epi.tile([128, TQ_MACRO], BF16, tag="ot")
                            nc.vector.tensor_copy(ot_b, ps_out)
                            dt_b = epi.tile([128, TQ_MACRO], BF16, tag="dt")
                            nc.vector.tensor_copy(dt_b, ps_den)

                            def _epi():
                                # denominator columns via one-hot matmuls:
                                # dcols[t, j] = dt_b[0, j*128+t]
                                dcols = ps_s.tile(
                                    [128, TQ_MACRO // 128], FP32, tag="sc"
                                )
                                for jj in range(TQ_MACRO // 128):
                                    nc.tensor.matmul(
                                        dcols[:, jj : jj + 1],
                                        dt_b[:, jj * 128 : (jj + 1) * 128],
                                        ident_bf[:, 0:1],
                                        start=True, stop=True,
                                    )
                                rcols = epi.tile(
                                    [128, TQ_MACRO // 128], FP32, tag="rcols"
                                )
                                nc.vector.reciprocal(rcols, dcols)
                                onat = epi.tile(
                                    [128, TQ_MACRO // 128, 128], FP32, tag="onat"
                                )
                                for jj in range(TQ_MACRO // 128):
                                    tp = ps_s.tile([128, 128], BF16, tag="sc")
                                    nc.tensor.transpose(
                                        tp,
                                        ot_b[:, jj * 128 : (jj + 1) * 128],
                                        ident_bf,
                                    )
                                    nc.vector.tensor_scalar_mul(
                                        onat[:, jj, :], tp, rcols[:, jj : jj + 1]
                                    )
                                    if jj % 4 == 3:
                                        hh = jj // 4
                                        nc.gpsimd.dma_start(
                                            out=out_dram[
                                                tq0 + hh * 512 : tq0 + (hh + 1) * 512,
                                                :,
                                            ].rearrange("(c p) d -> p c d", p=128),
                                            in_=onat[:, hh * 4 : (hh + 1) * 4, :],
                                        )

                            pending_epi.append(_epi)

                        return _pv

                    for j in range(N_CPAIRS):
                        # scores for chunks 2j, 2j+1 (bf16), one ps tile each
                        sc_tiles = []
                        for ci in range(2):
                            c = 2 * j + ci
                            ksl = kt[:, c * 128 : (c + 1) * 128]
                            ps_sc = ps_s.tile([128, TQ_MACRO], FP32, tag="sc")
                            for h in range(TQ_MACRO // 512):
                                sl = slice(h * 512, (h + 1) * 512)
                                qsl = slice(tq0 + h * 512, tq0 + (h + 1) * 512)
                                nc.tensor.matmul(
                                    ps_sc[:, sl], ksl, qt[:, qsl],
                                    start=True, stop=True,
                                )
                            sc_tiles.append(ps_sc)
                        if j == 2 and pending_epi:
                            pending_epi.pop(0)()
                        # PE: fp8 PV + den, 2 chunk-pairs behind
                        while len(pv_queue) > 1:
                            pv_queue.pop(0)()
                        # ScalarE: exp -> fp8 chunk-pair tile
                        pt2 = ptpool.tile([128, 2, TQ_MACRO], FP8)
                        for ci in range(2):
                            nc.scalar.activation(
                                pt2[:, ci, :], sc_tiles[ci],
                                mybir.ActivationFunctionType.Exp,
                                scale=INV_SCALE, bias=exp_bias,
                            )
                        pv_queue.append(_mk_pv(j, pt2))
                        # hide the NEXT pair's prologue in PE slack
                        if m >= 1:
                            pump(i + 1)
            while pv_queue:
                pv_queue.pop(0)()
            for f in pending_epi:
                f()
    _split_multi_waits(nc)
    return nc


def _get_nc():
    global _NC_CACHE
    if _NC_CACHE is None:
        _NC_CACHE = build_nc()
    return _NC_CACHE


def kernel(**inputs: np.ndarray) -> np.ndarray:
    x = np.ascontiguousarray(inputs["x"], dtype=np.float32)
    Wq = np.asarray(inputs["Wq"], dtype=np.float32)
    Wk = np.asarray(inputs["Wk"], dtype=np.float32)
    Wv = np.asarray(inputs["Wv"], dtype=np.float32)
    bq = np.asarray(inputs["bq"], dtype=np.float32)
    bk = np.asarray(inputs["bk"], dtype=np.float32)
    bv = np.asarray(inputs["bv"], dtype=np.float32)

    nc = _get_nc()

    in_maps = []
    for core in range(N_CORES):
        m = {}
        for i in range(PAIRS_PER_CORE):
            pair = core * PAIRS_PER_CORE + i
            b, g = pair // G, pair % G
            sl = slice(g * GS, (g + 1) * GS)
            m[f"x{i}"] = np.ascontiguousarray(x[b, :, sl])
            m[f"wq{i}"] = np.ascontiguousarray(Wq[g])
            m[f"wk{i}"] = np.ascontiguousarray(Wk[g])
            m[f"wv{i}"] = np.ascontiguousarray(Wv[g])
            m[f"bq{i}"] = np.ascontiguousarray(bq[g].reshape(1, GS))
            m[f"bk{i}"] = np.ascontiguousarray(bk[g].reshape(1, GS))
            m[f"bv{i}"] = np.ascontiguousarray(bv[g].reshape(1, GS))
        in_maps.append(m)

    global _LAST_IN_MAPS
    _LAST_IN_MAPS = in_maps

    from concourse.bass_utils import run_bass_kernel_spmd

    res = run_bass_kernel_spmd(nc, in_maps, list(range(N_CORES)))

    y = np.empty((B, T, F), dtype=np.float32)
    for core in range(N_CORES):
        for i in range(PAIRS_PER_CORE):
            pair = core * PAIRS_PER_CORE + i
            b, g = pair // G, pair % G
            y[b, :, g * GS : (g + 1) * GS] = res.results[core][f"y{i}"]
    return y
